# revision 3
# baseline (speedup 1.0000x reference)
"""BitLinear (activation int8-quant x ternary-weight linear) on 8 Trainium2 cores.

Strategy: tensor-parallel over W's output dim (column-parallel linear).
  - Host pre-transposes x -> xt [4096, 8192] (replicated to all 8 cores) and
    each core's W shard -> wt [4096, 1376].
  - Each core also gets xs = a 1/8 row-slice view of xt for the stats pass.
  - On device: per-core partial max|x| and sum|W| -> AllGather of [1,2]
    scalars -> every core derives identical act_scale / w_scale ->
    ternarize W shard into an SBUF-resident bf16 cache -> stream x tiles,
    quantize to integer-valued bf16, matmul (exact: products are small
    integers, f32 PSUM accumulate) -> scale by w_scale*act_scale -> out.
  - Host concatenates the 8 [8192, 1376] output shards.
"""

import numpy as np

import concourse.bass as bass
import concourse.mybir as mybir
import concourse.tile as tile
from concourse import bacc
from concourse.bass_utils import run_bass_kernel_spmd

F32 = mybir.dt.float32
BF16 = mybir.dt.bfloat16
AX = mybir.AxisListType
OP = mybir.AluOpType
ACTF = mybir.ActivationFunctionType

N_CORES = 8
MAGIC = 12582912.0  # 1.5 * 2**23: adding then subtracting rounds f32 to nearest-even int
Q_MAX = 127.0
R127 = float(np.float32(1.0) / np.float32(127.0))
EPS = 1e-8


def _build_nc(d_in, rows, out_sh, sb):
    """Build the SPMD bass program for one core.

    d_in:  contraction dim (4096)
    rows:  number of output rows = b*s (8192)
    out_sh: output columns per core (11008/8 = 1376)
    sb:    row super-block for quantization staging
    """
    kc = d_in // 128          # K chunks of 128
    xs_rows = d_in // N_CORES  # stats-pass rows of xt per core
    n_sb = rows // sb
    mb_per_sb = sb // 128
    n_slices = [(i, min(512, out_sh - i)) for i in range(0, out_sh, 512)]
    rn = float(np.float32(1.0 / (out_sh * N_CORES * d_in)))  # 1/numel(W)

    nc = bacc.Bacc(None, target_bir_lowering=False, debug=False)

    xt = nc.dram_tensor("xt", [d_in, rows], F32, kind="ExternalInput")
    xs = nc.dram_tensor("xs", [xs_rows, rows], F32, kind="ExternalInput")
    wt = nc.dram_tensor("wt", [d_in, out_sh], F32, kind="ExternalInput")
    out = nc.dram_tensor("out", [rows, out_sh], F32, kind="ExternalOutput")

    with tile.TileContext(nc) as tc:
        with (
            tc.tile_pool(name="const", bufs=1) as constp,
            tc.tile_pool(name="tw", bufs=1) as twp,
            tc.tile_pool(name="dram", bufs=1, space="DRAM") as dramp,
        ):
            tw_cache = twp.tile([128, kc, out_sh], BF16, name="tw_cache")
            bc = constp.tile([128, 4], F32, name="bc")  # cols: r_act, r_w, tot_scale

            # ---- Phase A: local stats + collective + scalar derivation ----
            with tc.tile_pool(name="stat", bufs=3) as statp:
                # max|x| over this core's xs slice
                xs_t = xs[:].rearrange("(c p) r -> c p r", p=128)
                xs_c = xs_rows // 128
                xchunk = min(2048, rows)
                n_xch = rows // xchunk
                pmax = statp.tile([128, xs_c * n_xch], F32, name="pmax", bufs=1)
                for i in range(xs_c):
                    for j in range(n_xch):
                        t = statp.tile([128, xchunk], F32, tag="xsld", name="xsld")
                        nc.sync.dma_start(t[:], xs_t[i, :, j * xchunk:(j + 1) * xchunk])
                        nc.vector.tensor_reduce(
                            pmax[:, i * n_xch + j: i * n_xch + j + 1], t[:],
                            axis=AX.X, op=OP.max, apply_absolute_value=True)
                rmax = statp.tile([128, 1], F32, name="rmax", bufs=1)
                nc.vector.tensor_reduce(rmax[:], pmax[:], axis=AX.X, op=OP.max)

                # sum|W| over this core's shard
                wt_t = wt[:].rearrange("(c p) o -> c p o", p=128)
                psum_w = statp.tile([128, kc], F32, name="psum_w", bufs=1)
                for c in range(kc):
                    t = statp.tile([128, out_sh], F32, tag="wsld", name="wsld")
                    nc.sync.dma_start(t[:], wt_t[c])
                    nc.vector.tensor_reduce(
                        psum_w[:, c:c + 1], t[:],
                        axis=AX.X, op=OP.add, apply_absolute_value=True)
                rsum = statp.tile([128, 1], F32, name="rsum", bufs=1)
                nc.vector.tensor_reduce(rsum[:], psum_w[:], axis=AX.X, op=OP.add)

                # cross-partition reduce on gpsimd -> [1,1] each
                mloc = statp.tile([1, 1], F32, name="mloc", bufs=1)
                sloc = statp.tile([1, 1], F32, name="sloc", bufs=1)
                nc.gpsimd.tensor_reduce(mloc[:], rmax[:], axis=AX.XYZWC, op=OP.max)
                nc.gpsimd.tensor_reduce(sloc[:], rsum[:], axis=AX.XYZWC, op=OP.add)

                # pack [1,2], AllGather -> [8,2]
                packed2 = statp.tile([1, 2], F32, name="packed2", bufs=1)
                nc.vector.tensor_copy(packed2[:, 0:1], mloc[:])
                nc.vector.tensor_copy(packed2[:, 1:2], sloc[:])
                cc_in = dramp.tile([1, 2], F32, name="cc_in")
                cc_out = dramp.tile([N_CORES, 2], F32, name="cc_out", addr_space="Shared")
                nc.sync.dma_start(cc_in[:], packed2[:])
                nc.gpsimd.collective_compute(
                    "AllGather", OP.bypass,
                    replica_groups=[list(range(N_CORES))],
                    ins=[cc_in[:].opt()], outs=[cc_out[:].opt()])

                gmax = statp.tile([1, N_CORES], F32, name="gmax", bufs=1)
                gsum = statp.tile([1, N_CORES], F32, name="gsum", bufs=1)
                nc.sync.dma_start(gmax[:], cc_out[:, 0:1].rearrange("a b -> b a"))
                nc.sync.dma_start(gsum[:], cc_out[:, 1:2].rearrange("a b -> b a"))

                m_g = statp.tile([1, 1], F32, name="m_g", bufs=1)
                s_g = statp.tile([1, 1], F32, name="s_g", bufs=1)
                nc.vector.tensor_reduce(m_g[:], gmax[:], axis=AX.X, op=OP.max)
                nc.vector.tensor_reduce(s_g[:], gsum[:], axis=AX.X, op=OP.add)

                # act_scale = max(m * (1/127), eps); w_scale = sum/N + eps
                a_sc = statp.tile([1, 1], F32, name="a_sc", bufs=1)
                w_sc = statp.tile([1, 1], F32, name="w_sc", bufs=1)
                nc.vector.tensor_scalar(a_sc[:], m_g[:], R127, EPS, OP.mult, OP.max)
                nc.vector.tensor_scalar(w_sc[:], s_g[:], rn, EPS, OP.mult, OP.add)

                # reciprocals (DVE) + one Newton step: r1 = r0*(2 - s*r0)
                def recip(name, s_ap):
                    r0 = statp.tile([1, 1], F32, name=name + "0", bufs=1)
                    nc.vector.reciprocal(r0[:], s_ap)
                    t1 = statp.tile([1, 1], F32, name=name + "t", bufs=1)
                    nc.vector.tensor_tensor(t1[:], s_ap, r0[:], op=OP.mult)
                    nc.vector.tensor_scalar(t1[:], t1[:], -1.0, 2.0, OP.mult, OP.add)
                    r1 = statp.tile([1, 1], F32, name=name + "1", bufs=1)
                    nc.vector.tensor_tensor(r1[:], r0[:], t1[:], op=OP.mult)
                    return r1

                r_act = recip("r_act", a_sc[:])
                r_w = recip("r_w", w_sc[:])
                tot = statp.tile([1, 1], F32, name="tot", bufs=1)
                nc.vector.tensor_tensor(tot[:], w_sc[:], a_sc[:], op=OP.mult)

                packed4 = statp.tile([1, 4], F32, name="packed4", bufs=1)
                nc.vector.tensor_copy(packed4[:, 0:1], r_act[:])
                nc.vector.tensor_copy(packed4[:, 1:2], r_w[:])
                nc.vector.tensor_copy(packed4[:, 2:3], tot[:])
                nc.vector.tensor_copy(packed4[:, 3:4], tot[:])
                nc.gpsimd.partition_broadcast(bc[:], packed4[:])

            # ---- Phase B: ternarize W shard into SBUF bf16 cache ----
            with (
                tc.tile_pool(name="wld", bufs=2) as wldp,
                tc.tile_pool(name="xio", bufs=3) as xiop,
                tc.tile_pool(name="qx", bufs=2) as qxp,
                tc.tile_pool(name="ot", bufs=2) as otp,
                tc.tile_pool(name="ps", bufs=2, space="PSUM") as psp,
            ):
                wt_t = wt[:].rearrange("(c p) o -> c p o", p=128)
                for c in range(kc):
                    wtile = wldp.tile([128, out_sh], F32, tag="wtile", name="wtile")
                    nc.sync.dma_start(wtile[:], wt_t[c])
                    wtmp = wldp.tile([128, out_sh], F32, tag="wtmp", name="wtmp")
                    nc.scalar.activation(wtmp[:], wtile[:], ACTF.Copy,
                                         bias=MAGIC, scale=bc[:, 1:2])
                    nc.vector.tensor_scalar(wtmp[:], wtmp[:], MAGIC, 1.0,
                                            OP.subtract, OP.min)
                    nc.vector.tensor_scalar_max(tw_cache[:, c, :], wtmp[:], -1.0)

                # ---- Phase C: quantize x + matmul + scale + store ----
                xt_t = xt[:].rearrange("(c p) r -> c p r", p=128)
                out_t = out[:].rearrange("(m p) o -> m p o", p=128)
                for s in range(n_sb):
                    qx = qxp.tile([128, kc, sb], BF16, tag="qx", name="qx")
                    for c in range(kc):
                        xtile = xiop.tile([128, sb], F32, tag="xtile", name="xtile")
                        nc.sync.dma_start(xtile[:], xt_t[c, :, s * sb:(s + 1) * sb])
                        xtmp = xiop.tile([128, sb], F32, tag="xtmp", name="xtmp")
                        nc.scalar.activation(xtmp[:], xtile[:], ACTF.Copy,
                                             bias=MAGIC, scale=bc[:, 0:1])
                        nc.vector.tensor_scalar(xtmp[:], xtmp[:], MAGIC, Q_MAX,
                                                OP.subtract, OP.min)
                        nc.vector.tensor_scalar_max(qx[:, c, :], xtmp[:], -Q_MAX)
                    for mb in range(mb_per_sb):
                        ps = psp.tile([128, out_sh], F32, tag="ps", name="ps")
                        for (n0, nsz) in n_slices:
                            for c in range(kc):
                                nc.tensor.matmul(
                                    ps[:, n0:n0 + nsz],
                                    qx[:, c, mb * 128:(mb + 1) * 128],
                                    tw_cache[:, c, n0:n0 + nsz],
                                    start=(c == 0), stop=(c == kc - 1))
                        ot = otp.tile([128, out_sh], F32, tag="ot", name="ot")
                        nc.scalar.activation(ot[:], ps[:], ACTF.Copy,
                                             bias=0.0, scale=bc[:, 2:3])
                        nc.sync.dma_start(out_t[s * mb_per_sb + mb], ot[:])

    nc.compile()
    return nc


_NC_CACHE = {}


def _get_nc(d_in, rows, out_sh, sb):
    key = (d_in, rows, out_sh, sb)
    if key not in _NC_CACHE:
        _NC_CACHE[key] = _build_nc(d_in, rows, out_sh, sb)
    return _NC_CACHE[key]


def _prep(x, W):
    x = np.asarray(x)
    W = np.asarray(W)
    assert x.dtype == np.float32 and W.dtype == np.float32
    b, s, d_in = x.shape
    d_out = W.shape[0]
    rows = b * s
    out_sh = d_out // N_CORES
    sb = 512

    xt = np.ascontiguousarray(x.reshape(rows, d_in).T)  # [d_in, rows]
    xs_rows = d_in // N_CORES

    in_maps = []
    for c in range(N_CORES):
        in_maps.append({
            "xt": xt,
            "xs": xt[c * xs_rows:(c + 1) * xs_rows],
            "wt": np.ascontiguousarray(W[c * out_sh:(c + 1) * out_sh, :].T),
        })

    nc = _get_nc(d_in, rows, out_sh, sb)

    def assemble(results):
        out = np.concatenate([results[c]["out"] for c in range(N_CORES)], axis=1)
        return out.reshape(b, s, d_out)

    return nc, in_maps, assemble


def kernel(x, W):
    nc, in_maps, assemble = _prep(x, W)
    res = run_bass_kernel_spmd(nc, in_maps, core_ids=list(range(N_CORES)))
    return assemble(res.results)
